# revision 23
# baseline (speedup 1.0000x reference)
"""Trainium2 Bass kernel: causal multi-head group attention (GQA) with RoPE.

Full-input contract: kernel(**inputs) takes the unsharded inputs and returns
the full output. Internally shards across 8 NeuronCores:
  core c -> (batch b = c // 4, head-group g = c % 4)
Each core computes 4 q heads + their single kv group end-to-end (QKV proj,
RoPE, causal flash-style attention, row-parallel out-proj partial). The host
unshard step sums the 4 head-group partials per batch and adds the output
bias.

The per-core program is a software pipeline over 512-token passes: each pass
projects q/k/v for its token window (bf16 operands, f32 PSUM accumulation),
applies RoPE, runs causal attention for that query chunk against all keys so
far, and defers the out-proj for those rows into the next pass so its PE work
never waits on the normalize chain. Matmul operands are bf16 (full PE rate at
any width); softmax numerator accumulation stays f32.

Engine placement keeps PE the only near-saturated engine:
  PE   - all matmuls, with attention scores emitted two k-tiles ahead of the
         P@V accumulation so PV's wait on exp never idles the PE queue
  ACT  - exp(softmax) + all PSUM->SBUF projection copies + half the out-proj
         staging copies
  DVE  - softmax numerator accumulation (f32), masking, reciprocal,
         y-normalize, other half of out-proj staging
  Pool - RoPE (fused (swap*sgn+t)*sin) + softmax-denominator partition
         all-reduce (keeps the PE queue free of ones-matmuls)
  DMA  - x/weight streaming (chunk-interleaved in consumption order), rope
         half-swaps, output writeback
"""

import os
import sys
from contextlib import ExitStack, nullcontext
from math import sqrt

for _p in ("/opt/trn_rl_repo", "/root/.axon_site/_ro/trn_rl_repo"):
    if os.path.isdir(_p) and _p not in sys.path:
        sys.path.insert(0, _p)

import numpy as np
import concourse.bacc as bacc
import concourse.tile as tile
import concourse.mybir as mybir
from concourse import bass_isa
from concourse.bass_utils import run_bass_kernel_spmd

F32 = mybir.dt.float32
BF16 = mybir.dt.bfloat16
NP_BF16 = mybir.dt.np(BF16)
EXP = mybir.ActivationFunctionType.Exp
MULT = mybir.AluOpType.mult
ADD = mybir.AluOpType.add

N_CORES = 8
TP = 4            # head-group parallel degree (within one batch element)
BATCH = 2
D = 128           # head dim
NHL = 4           # q heads per core
ROPE_BASE = 10000.0

# Full-problem config
S_FULL = 2048     # context length
E_FULL = 2048     # model dim


def build_program(S, E, QC=512, PW=512, n_cores=N_CORES, reps=1, staggered=True):
    """Emit the per-core SPMD program. QC: q-chunk width (attention moving dim),
    PW: phase-1 pass width over the sequence (must equal QC for the pipelined
    schedule). reps>1 wraps the whole body in an on-device For_i loop (timing
    builds only)."""
    assert PW == QC
    EC = E // 128     # contraction chunks over model dim
    NKI = S // 128    # k tiles
    NSP = S // PW     # passes
    B = QC - 128      # mask-table base offset
    MW = 2 * QC - 128 # mask-table width
    scale = 1.0 / sqrt(D)

    nc = bacc.Bacc("TRN2", target_bir_lowering=False, debug=False,
                   num_devices=n_cores)

    # host-preswizzled: every tensor is already in its SBUF tile layout so
    # each loads with ONE contiguous-line DMA
    xh = nc.dram_tensor("xh", [128, S * E // 128], BF16, kind="ExternalInput").ap()
    Wq = nc.dram_tensor("Wq", [128, EC * NHL * D], BF16, kind="ExternalInput").ap()
    Wk = nc.dram_tensor("Wk", [128, EC * D], BF16, kind="ExternalInput").ap()
    Wv = nc.dram_tensor("Wv", [128, EC * D], BF16, kind="ExternalInput").ap()
    Wo = nc.dram_tensor("Wo", [128, NHL * E], BF16, kind="ExternalInput").ap()
    sinT = nc.dram_tensor("sinT", [D, S], BF16, kind="ExternalInput").ap()
    cst = nc.dram_tensor("cst", [128, 257], BF16, kind="ExternalInput").ap()
    out = nc.dram_tensor("out", [S, E], BF16, kind="ExternalOutput").ap()

    with tile.TileContext(nc) as tc, \
         (tc.For_i(0, reps, 1, staggered_reset=staggered) if reps > 1 else nullcontext()), \
         nc.allow_low_precision(reason="bf16 internal compute, f32 psum"), \
         ExitStack() as top:
        pers = top.enter_context(tc.tile_pool(name="pers", bufs=1))
        kT_sb = pers.tile([128, S], BF16, name="kT_sb")
        v_sb = [pers.tile([128, D], BF16, tag=f"v{i}", name=f"v{i}")
                for i in range(NKI)]
        wq = pers.tile([128, EC * NHL * D], BF16, tag="wq", name="wq")
        wk = pers.tile([128, EC * D], BF16, tag="wk", name="wk")
        wv = pers.tile([128, EC * D], BF16, tag="wv", name="wv")
        wo = pers.tile([128, NHL * E], BF16, tag="wo", name="wo")
        sin_sb = pers.tile([128, S], BF16, tag="sin", name="sin_sb")
        cst_sb = pers.tile([128, 257], BF16, tag="cst", name="cst_sb")
        mw_sb = cst_sb[:, 0:128]
        ident_sb = cst_sb[:, 128:256]
        sgn_sb = cst_sb[:, 256:257]

        # PSUM pool: per-tag buffer counts sum to exactly 8 banks.
        psum = top.enter_context(tc.tile_pool(name="psum", bufs=1, space="PSUM"))
        # Rotating SBUF pools.
        xpool = top.enter_context(tc.tile_pool(name="xt", bufs=4))
        qpool = top.enter_context(tc.tile_pool(name="qp", bufs=2))
        ypool = top.enter_context(tc.tile_pool(name="yp", bufs=2))
        opool = top.enter_context(tc.tile_pool(name="osb", bufs=2))
        stg = top.enter_context(tc.tile_pool(name="stg", bufs=2))
        hot = top.enter_context(tc.tile_pool(name="hot", bufs=1))

        # ---- startup DMAs: small consts first, then chunk-interleaved
        # weight / x streaming in roughly the order the PE consumes them.
        nc.sync.dma_start(cst_sb[:], cst[:])

        HB = (EC // 2) * PW  # elements per x half-tile

        def load_x_half(xt_tile, sp, half):
            off = (sp * 2 + half) * HB
            nc.sync.dma_start(xt_tile[:], xh[:, off:off + HB])

        # pass-0 x and weights, ordered so the first projection chains can
        # start as soon as their operands land
        xt0a = xpool.tile([128, HB], BF16, tag="xt", bufs=4, name="xt0a")
        xt0b = xpool.tile([128, HB], BF16, tag="xt", name="xt0b")
        WQH = EC * D  # one head's worth of wq columns (head-major layout)
        # every projection chain contracts over the FULL x pass, so x must
        # finish streaming before ANY chain completes: x goes first, then
        # the weights in chain-consumption order
        nc.sync.dma_start(wk[:], Wk[:])
        nc.sync.dma_start(xt0a[:, 0:HB // 2], xh[:, 0:HB // 2])
        nc.sync.dma_start(xt0a[:, HB // 2:HB], xh[:, HB // 2:HB])
        nc.sync.dma_start(xt0b[:, 0:HB // 2], xh[:, HB:HB + HB // 2])
        nc.sync.dma_start(xt0b[:, HB // 2:HB], xh[:, HB + HB // 2:2 * HB])
        nc.sync.dma_start(wv[:], Wv[:])
        for hh in range(NHL):
            nc.sync.dma_start(wq[:, hh * WQH:(hh + 1) * WQH],
                              Wq[:, hh * WQH:(hh + 1) * WQH])
        nc.sync.dma_start(sin_sb[:], sinT[:])
        nc.sync.dma_start(wo[:], Wo[:])

        def out_proj_chunk(sp, loc):
            """Row-parallel Wo partial for row-block loc of pass sp."""
            yT = yT_of[sp]
            if True:
                si = (QC // 128) * sp + loc
                osb = opool.tile([128, E], BF16, tag="osb", name=f"osb{si}")
                for nj in range(E // 512):
                    ops = psum.tile([128, 512], F32, tag="proj", bufs=2,
                                    name=f"ops{si}_{nj}")
                    for h in range(NHL):
                        nc.tensor.matmul(
                            ops[:],
                            yT[:, QC * h + 128 * loc:QC * h + 128 * (loc + 1)],
                            wo[:, E * h + 512 * nj:E * h + 512 * (nj + 1)],
                            start=(h == 0), stop=(h == NHL - 1))
                    if nj % 2 == 0:
                        nc.scalar.copy(osb[:, 512 * nj:512 * (nj + 1)], ops[:])
                    else:
                        nc.vector.tensor_copy(osb[:, 512 * nj:512 * (nj + 1)],
                                              ops[:])
                    if nj == 1:
                        nc.sync.dma_start(
                            out[128 * si:128 * (si + 1), 0:1024],
                            osb[:, 0:1024])
                nc.sync.dma_start(out[128 * si:128 * (si + 1), 1024:E],
                                  osb[:, 1024:E])

        def denom_norm(sp, h, rs, yps, yT):
            """Softmax denominator + normalize for head h of pass sp."""
            rsa = hot.tile([128, QC], F32, tag="rsa", bufs=2,
                           name=f"rsa{h}_{sp}")
            nc.gpsimd.partition_all_reduce(rsa[:], rs[:], 128,
                                           bass_isa.ReduceOp.add)
            rinv = hot.tile([128, QC], F32, tag="rinv", bufs=2,
                            name=f"rinv{h}_{sp}")
            nc.vector.reciprocal(rinv[:], rsa[:])
            nc.vector.tensor_mul(yT[:, QC * h:QC * (h + 1)], yps[:], rinv[:])

        def rope_swap(t_ap, name):
            """Issue the half-swap DMAs for rope; DVE math comes later."""
            tmp = stg.tile([128, PW], BF16, tag="ropetmp", bufs=2,
                           name=f"rt{name}")
            nc.sync.dma_start(tmp[0:64, :], t_ap[64:128, :])
            nc.sync.dma_start(tmp[64:128, :], t_ap[0:64, :])
            return tmp

        def rope_apply(tmp, t_ap, s0):
            """Buggy-faithful rope in place: t = (t + sgn*swap_half(t)) * sin."""
            nc.vector.scalar_tensor_tensor(tmp, tmp, sgn_sb, t_ap, MULT, ADD)
            nc.vector.tensor_mul(t_ap, tmp, sin_sb[:, s0:s0 + PW])

        def proj_group(sp, g, qT, xa, xb, ropes):
            """Projection-chain group g of pass sp: g0=k, g1=v(+transpose),
            g2=q0/q1, g3=q2/q3. Emitted as PE filler inside the previous
            pass's attention. Rope swap-DMAs issue here; the DVE math is
            deferred into `ropes` and applied at the end of the attention
            stretch."""
            s0 = PW * sp

            def xsl(e):
                t = xa if e < EC // 2 else xb
                return t[:, PW * (e % (EC // 2)):PW * (e % (EC // 2) + 1)]

            if g == 0:
                ps = psum.tile([128, PW], F32, tag="proj", bufs=2,
                               name=f"psk{sp}")
                for e in range(EC):
                    nc.tensor.matmul(ps[:], wk[:, D * e:D * (e + 1)], xsl(e),
                                     start=(e == 0), stop=(e == EC - 1))
                t_ap = kT_sb[:, s0:s0 + PW]
                nc.scalar.copy(t_ap, ps[:])
                ropes.append((rope_swap(t_ap, f"k{sp}")[:], t_ap, s0))
            elif g == 1:
                ps = psum.tile([128, PW], F32, tag="proj", bufs=2,
                               name=f"psv{sp}")
                for e in range(EC):
                    nc.tensor.matmul(ps[:], wv[:, D * e:D * (e + 1)], xsl(e),
                                     start=(e == 0), stop=(e == EC - 1))
                vstage = stg.tile([128, PW], BF16, tag="vstage", bufs=1,
                                  name=f"vst{sp}")
                nc.scalar.copy(vstage[:], ps[:])
                for j in range(PW // 128):
                    vt_ps = psum.tile([128, 128], BF16, tag="proj", bufs=2,
                                      name=f"vtr{sp}_{j}")
                    nc.tensor.transpose(vt_ps[:],
                                        vstage[:, 128 * j:128 * (j + 1)],
                                        ident_sb)
                    nc.scalar.copy(v_sb[s0 // 128 + j][:], vt_ps[:])
            else:
                for h in (2 * (g - 2), 2 * (g - 2) + 1):
                    ps = psum.tile([128, PW], F32, tag="proj", bufs=2,
                                   name=f"psq{sp}_{h}")
                    for e in range(EC):
                        nc.tensor.matmul(
                            ps[:],
                            wq[:, EC * D * h + D * e:EC * D * h + D * (e + 1)],
                            xsl(e), start=(e == 0), stop=(e == EC - 1))
                    nc.scalar.copy(qT[:, PW * h:PW * (h + 1)], ps[:])
                if g == 3:
                    # all heads swap the same partition halves: one batched
                    # DMA pair covers the whole 4-head tile
                    tmpq = stg.tile([128, NHL * PW], BF16, tag="ropetmpq",
                                    bufs=2, name=f"rtq{sp}")
                    nc.sync.dma_start(tmpq[0:64, :], qT[64:128, :])
                    nc.sync.dma_start(tmpq[64:128, :], qT[0:64, :])
                    for h in range(NHL):
                        ropes.append((tmpq[:, PW * h:PW * (h + 1)],
                                      qT[:, PW * h:PW * (h + 1)], s0))

        # ---- pass-0 projections upfront (startup); later passes are
        # emitted as PE filler inside the previous pass's attention
        yT_of = {}
        qT_cur = qpool.tile([128, NHL * PW], BF16, tag="qT", name="qT0")
        ropes0 = []
        for g in range(4):
            proj_group(0, g, qT_cur, xt0a, xt0b, ropes0)
        for args in ropes0:
            rope_apply(*args)

        xt_cur = (xt0a, xt0b)
        for sp in range(NSP):
            xa, xb = xt_cur
            qT = qT_cur
            ropes = []
            if sp + 1 < NSP:
                xt1a = xpool.tile([128, (EC // 2) * PW], BF16, tag="xt",
                                  name=f"xt{sp + 1}a")
                xt1b = xpool.tile([128, (EC // 2) * PW], BF16, tag="xt",
                                  name=f"xt{sp + 1}b")
                load_x_half(xt1a, sp + 1, 0)
                load_x_half(xt1b, sp + 1, 1)
                xt_cur = (xt1a, xt1b)
                qT_cur = qpool.tile([128, NHL * PW], BF16, tag="qT",
                                    name=f"qT{sp + 1}")

            # ---- causal attention for query chunk qj == sp
            yT = ypool.tile([128, NHL * QC], BF16, tag="yT", name=f"yT{sp}")
            yT_of[sp] = yT
            nki_hi = 4 * (sp + 1)
            prev = None  # deferred denom+normalize of the previous head
            for h in range(NHL):
                qh = qT[:, QC * h:QC * (h + 1)]
                yps = psum.tile([128, QC], F32, tag="yps", bufs=2,
                                name=f"yps{h}_{sp}")
                rs = hot.tile([128, QC], BF16, tag="rs", bufs=2,
                              name=f"rs{h}_{sp}")
                pend = []  # scores run a k-tile pair ahead of P@V
                pt0 = None
                for kp in range(0, nki_hi, 2):
                    # two k-tiles share a 2-bank PSUM tile and ONE exp op,
                    # halving the ACT access-latency overhead
                    st = psum.tile([128, 2 * QC], F32, tag="st", bufs=2,
                                   name=f"st{h}_{sp}_{kp}")
                    pt = hot.tile([128, 2 * QC], BF16, tag="pt", bufs=3,
                                  name=f"pt{h}_{sp}_{kp}")[:]
                    qlos = []
                    for j in (0, 1):
                        ki = kp + j
                        off = 128 * ki - QC * sp
                        qlo = max(0, off)  # fully-masked columns are skipped
                        qlos.append(qlo)
                        nc.tensor.matmul(
                            st[:, QC * j + qlo:QC * (j + 1)],
                            kT_sb[:, 128 * ki:128 * (ki + 1)],
                            qh[:, qlo:QC], start=True, stop=True)
                    # exp spans both slabs; the gap [QC:QC+qlo1) is stale
                    # PSUM whose exp lands in masked-out, never-read columns
                    nc.scalar.activation(pt[:, qlos[0]:2 * QC],
                                         st[:, qlos[0]:2 * QC], EXP,
                                         scale=scale)
                    for j in (0, 1):
                        ki = kp + j
                        off = 128 * ki - QC * sp
                        qlo = qlos[j]
                        ptj = pt[:, QC * j:QC * (j + 1)]
                        if off >= 0:
                            nc.vector.tensor_mul(ptj[:, qlo:qlo + 128],
                                                 ptj[:, qlo:qlo + 128],
                                                 mw_sb)
                        if ki == 0:
                            pt0 = ptj
                        elif ki == 1:
                            if qlo > 0:
                                nc.vector.tensor_copy(rs[:, 0:qlo],
                                                      pt0[:, 0:qlo])
                            nc.vector.tensor_add(rs[:, qlo:QC],
                                                 pt0[:, qlo:QC],
                                                 ptj[:, qlo:QC])
                        else:
                            nc.vector.tensor_add(rs[:, qlo:QC],
                                                 rs[:, qlo:QC],
                                                 ptj[:, qlo:QC])
                        pend.append((ki, qlo, ptj))
                    while len(pend) > 4:
                        k0, q0, p0 = pend.pop(0)
                        nc.tensor.matmul(yps[:, q0:QC], v_sb[k0][:],
                                         p0[:, q0:QC], start=(k0 == 0),
                                         stop=(k0 == nki_hi - 1))
                    if kp == 0 and prev is not None:
                        denom_norm(sp, *prev)
                        prev = None
                for k0, q0, p0 in pend:
                    nc.tensor.matmul(yps[:, q0:QC], v_sb[k0][:], p0[:, q0:QC],
                                     start=(k0 == 0), stop=(k0 == nki_hi - 1))
                prev = (h, rs, yps, yT)
                # PE filler between heads: prior pass's out-proj rows and the
                # next pass's projection chains
                if sp > 0:
                    out_proj_chunk(sp - 1, h)
                if sp + 1 < NSP:
                    proj_group(sp + 1, h, qT_cur, xt_cur[0], xt_cur[1], ropes)
            denom_norm(sp, *prev)
            for args in ropes:
                rope_apply(*args)

        for loc in range(QC // 128):
            out_proj_chunk(NSP - 1, loc)

    nc.compile()
    return nc


def make_consts(S, QC=512):
    """Host-precomputed constant tensors (rope table, causal mask, sign, identity)."""
    rope_dim = D // 2
    j = np.arange(rope_dim, dtype=np.float64)
    thetas = 1.0 / ROPE_BASE ** (2.0 * j / rope_dim)
    positions = np.arange(S, dtype=np.float64)
    angles = positions[:, None] * thetas[None, :]
    sin = np.sin(np.concatenate([angles, angles], axis=1)).astype(np.float32)  # [S, D]
    sinT = np.ascontiguousarray(sin.T)                                          # [D, S]
    sgn = np.where(np.arange(D) < rope_dim, -1.0, 1.0).astype(np.float32)

    B = QC - 128
    MW = 2 * QC - 128
    k_idx = np.arange(128)[:, None]
    c_idx = np.arange(MW)[None, :]
    mw = (k_idx <= (c_idx - B)).astype(np.float32)

    cst = np.concatenate([mw[:, B:B + 128], np.eye(128, dtype=np.float32),
                          sgn.reshape(128, 1)], axis=1)
    return {
        "sinT": np.ascontiguousarray(sinT.astype(NP_BF16)),
        "cst": np.ascontiguousarray(cst).astype(NP_BF16),
    }


def _swz_w(W):
    """[A*128, N] -> SBUF layout [128, A*N] (chunk-major along free dim)."""
    A = W.shape[0] // 128
    return np.ascontiguousarray(
        W.reshape(A, 128, W.shape[1]).transpose(1, 0, 2).reshape(128, -1)
    ).astype(NP_BF16)


def _swz_x(xb, S, E, PW):
    """[S, E] -> [128, NSP*2*(EC/2)*PW]: per pass-half, e-chunk-major tiles."""
    NSP, EC = S // PW, E // 128
    a = xb.reshape(NSP, PW, EC, 128).transpose(3, 0, 2, 1)   # [128,NSP,EC,PW]
    a = a.reshape(128, NSP, 2, (EC // 2) * PW)
    return np.ascontiguousarray(a.reshape(128, -1)).astype(NP_BF16)


def make_in_maps(x, Wq, Wk, Wv, Wo, S, E, QC=512, bo=None):
    """Shard full inputs into the 8 per-core input maps (bf16 on-device)."""
    consts = make_consts(S, QC)
    EC_ = E // 128
    in_maps = []
    for c in range(N_CORES):
        b, g = c // TP, c % TP
        m = dict(consts)
        m["xh"] = _swz_x(x[b], S, E, QC)
        wqg = Wq[:, NHL * D * g:NHL * D * (g + 1)]            # [E, NHL*D]
        wqg = wqg.reshape(EC_, 128, NHL, D).transpose(1, 2, 0, 3)  # [128,h,e,d]
        m["Wq"] = np.ascontiguousarray(
            wqg.reshape(128, -1)).astype(NP_BF16)
        m["Wk"] = _swz_w(Wk[:, D * g:D * (g + 1)])
        m["Wv"] = _swz_w(Wv[:, D * g:D * (g + 1)])
        m["Wo"] = _swz_w(Wo[NHL * D * g:NHL * D * (g + 1), :])
        in_maps.append(m)
    return in_maps


_CACHE = {}


def _compiled_full():
    if "nc" not in _CACHE:
        _CACHE["nc"] = build_program(S_FULL, E_FULL)
    return _CACHE["nc"]


def kernel(x, Wq, Wk, Wv, Wo, bo):
    nc = _compiled_full()
    in_maps = make_in_maps(x, Wq, Wk, Wv, Wo, S_FULL, E_FULL, bo=bo)
    res = run_bass_kernel_spmd(nc, in_maps, list(range(N_CORES)))
    # unshard the row-parallel out-proj: sum the 4 head-group partials,
    # then add the output bias (host side of the reduce)
    out = np.zeros((BATCH, S_FULL, E_FULL), np.float32)
    for c in range(N_CORES):
        out[c // TP] += res.results[c]["out"].astype(np.float32)
    out += bo.astype(np.float32)[None, None, :]
    return out


# revision 24
# speedup vs baseline: 1.0170x; 1.0170x over previous
"""Trainium2 Bass kernel: causal multi-head group attention (GQA) with RoPE.

Full-input contract: kernel(**inputs) takes the unsharded inputs and returns
the full output. Internally shards across 8 NeuronCores:
  core c -> (batch b = c // 4, head-group g = c % 4)
Each core computes 4 q heads + their single kv group end-to-end (QKV proj,
RoPE, causal flash-style attention, row-parallel out-proj partial). The host
unshard step sums the 4 head-group partials per batch and adds the output
bias.

The per-core program is a software pipeline over 512-token passes: each pass
projects q/k/v for its token window (bf16 operands, f32 PSUM accumulation),
applies RoPE, runs causal attention for that query chunk against all keys so
far, and defers the out-proj for those rows into the next pass so its PE work
never waits on the normalize chain. Matmul operands are bf16 (full PE rate at
any width); softmax numerator accumulation stays f32.

Engine placement keeps PE the only near-saturated engine:
  PE   - all matmuls, with attention scores emitted two k-tiles ahead of the
         P@V accumulation so PV's wait on exp never idles the PE queue
  ACT  - exp(softmax) + all PSUM->SBUF projection copies + half the out-proj
         staging copies
  DVE  - softmax numerator accumulation (f32), masking, reciprocal,
         y-normalize, other half of out-proj staging
  Pool - RoPE (fused (swap*sgn+t)*sin) + softmax-denominator partition
         all-reduce (keeps the PE queue free of ones-matmuls)
  DMA  - x/weight streaming (chunk-interleaved in consumption order), rope
         half-swaps, output writeback
"""

import os
import sys
from contextlib import ExitStack, nullcontext
from math import sqrt

for _p in ("/opt/trn_rl_repo", "/root/.axon_site/_ro/trn_rl_repo"):
    if os.path.isdir(_p) and _p not in sys.path:
        sys.path.insert(0, _p)

import numpy as np
import concourse.bacc as bacc
import concourse.tile as tile
import concourse.mybir as mybir
from concourse import bass_isa
from concourse.bass_utils import run_bass_kernel_spmd

F32 = mybir.dt.float32
BF16 = mybir.dt.bfloat16
NP_BF16 = mybir.dt.np(BF16)
EXP = mybir.ActivationFunctionType.Exp
MULT = mybir.AluOpType.mult
ADD = mybir.AluOpType.add

N_CORES = 8
TP = 4            # head-group parallel degree (within one batch element)
BATCH = 2
D = 128           # head dim
NHL = 4           # q heads per core
ROPE_BASE = 10000.0

# Full-problem config
S_FULL = 2048     # context length
E_FULL = 2048     # model dim


def build_program(S, E, QC=512, PW=512, n_cores=N_CORES, reps=1, staggered=True):
    """Emit the per-core SPMD program. QC: q-chunk width (attention moving dim),
    PW: phase-1 pass width over the sequence (must equal QC for the pipelined
    schedule). reps>1 wraps the whole body in an on-device For_i loop (timing
    builds only)."""
    assert PW == QC
    EC = E // 128     # contraction chunks over model dim
    NKI = S // 128    # k tiles
    NSP = S // PW     # passes
    B = QC - 128      # mask-table base offset
    MW = 2 * QC - 128 # mask-table width
    scale = 1.0 / sqrt(D)

    nc = bacc.Bacc("TRN2", target_bir_lowering=False, debug=False,
                   num_devices=n_cores)

    # host-preswizzled: every tensor is already in its SBUF tile layout so
    # each loads with ONE contiguous-line DMA
    xh = nc.dram_tensor("xh", [128, S * E // 128], BF16, kind="ExternalInput").ap()
    Wq = nc.dram_tensor("Wq", [128, EC * NHL * D], BF16, kind="ExternalInput").ap()
    Wk = nc.dram_tensor("Wk", [128, EC * D], BF16, kind="ExternalInput").ap()
    Wv = nc.dram_tensor("Wv", [128, EC * D], BF16, kind="ExternalInput").ap()
    Wo = nc.dram_tensor("Wo", [128, NHL * E], BF16, kind="ExternalInput").ap()
    sinT = nc.dram_tensor("sinT", [D, S], BF16, kind="ExternalInput").ap()
    cst = nc.dram_tensor("cst", [128, 257], BF16, kind="ExternalInput").ap()
    out = nc.dram_tensor("out", [S, E], BF16, kind="ExternalOutput").ap()

    with tile.TileContext(nc) as tc, \
         (tc.For_i(0, reps, 1, staggered_reset=staggered) if reps > 1 else nullcontext()), \
         nc.allow_low_precision(reason="bf16 internal compute, f32 psum"), \
         ExitStack() as top:
        pers = top.enter_context(tc.tile_pool(name="pers", bufs=1))
        kT_sb = pers.tile([128, S], BF16, name="kT_sb")
        v_sb = [pers.tile([128, D], BF16, tag=f"v{i}", name=f"v{i}")
                for i in range(NKI)]
        wq = pers.tile([128, EC * NHL * D], BF16, tag="wq", name="wq")
        wk = pers.tile([128, EC * D], BF16, tag="wk", name="wk")
        wv = pers.tile([128, EC * D], BF16, tag="wv", name="wv")
        wo = pers.tile([128, NHL * E], BF16, tag="wo", name="wo")
        sin_sb = pers.tile([128, S], BF16, tag="sin", name="sin_sb")
        cst_sb = pers.tile([128, 257], BF16, tag="cst", name="cst_sb")
        mw_sb = cst_sb[:, 0:128]
        ident_sb = cst_sb[:, 128:256]
        sgn_sb = cst_sb[:, 256:257]

        # PSUM pool: per-tag buffer counts sum to exactly 8 banks.
        psum = top.enter_context(tc.tile_pool(name="psum", bufs=1, space="PSUM"))
        # Rotating SBUF pools.
        xpool = top.enter_context(tc.tile_pool(name="xt", bufs=4))
        qpool = top.enter_context(tc.tile_pool(name="qp", bufs=2))
        ypool = top.enter_context(tc.tile_pool(name="yp", bufs=2))
        opool = top.enter_context(tc.tile_pool(name="osb", bufs=2))
        stg = top.enter_context(tc.tile_pool(name="stg", bufs=2))
        hot = top.enter_context(tc.tile_pool(name="hot", bufs=1))

        # ---- startup DMAs: small consts first, then chunk-interleaved
        # weight / x streaming in roughly the order the PE consumes them.
        nc.sync.dma_start(cst_sb[:], cst[:])

        HB = (EC // 2) * PW  # elements per x half-tile

        def load_x_half(xt_tile, sp, half):
            off = (sp * 2 + half) * HB
            nc.sync.dma_start(xt_tile[:], xh[:, off:off + HB])

        # pass-0 x and weights, ordered so the first projection chains can
        # start as soon as their operands land
        xt0a = xpool.tile([128, HB], BF16, tag="xt", bufs=4, name="xt0a")
        xt0b = xpool.tile([128, HB], BF16, tag="xt", name="xt0b")
        WQH = EC * D  # one head's worth of wq columns (head-major layout)
        # every projection chain contracts over the FULL x pass, so x must
        # finish streaming before ANY chain completes: x goes first, then
        # the weights in chain-consumption order
        nc.sync.dma_start(wk[:], Wk[:])
        nc.sync.dma_start(xt0a[:, 0:HB // 2], xh[:, 0:HB // 2])
        nc.sync.dma_start(xt0a[:, HB // 2:HB], xh[:, HB // 2:HB])
        nc.sync.dma_start(xt0b[:, 0:HB // 2], xh[:, HB:HB + HB // 2])
        nc.sync.dma_start(xt0b[:, HB // 2:HB], xh[:, HB + HB // 2:2 * HB])
        nc.sync.dma_start(wv[:], Wv[:])
        for hh in range(NHL):
            nc.sync.dma_start(wq[:, hh * WQH:(hh + 1) * WQH],
                              Wq[:, hh * WQH:(hh + 1) * WQH])
        nc.sync.dma_start(sin_sb[:], sinT[:])
        nc.sync.dma_start(wo[:], Wo[:])

        def out_proj_chunk(sp, loc):
            """Row-parallel Wo partial for row-block loc of pass sp."""
            yT = yT_of[sp]
            if True:
                si = (QC // 128) * sp + loc
                osb = opool.tile([128, E], BF16, tag="osb", name=f"osb{si}")
                for nj in range(E // 512):
                    ops = psum.tile([128, 512], F32, tag="proj", bufs=2,
                                    name=f"ops{si}_{nj}")
                    for h in range(NHL):
                        nc.tensor.matmul(
                            ops[:],
                            yT[:, QC * h + 128 * loc:QC * h + 128 * (loc + 1)],
                            wo[:, E * h + 512 * nj:E * h + 512 * (nj + 1)],
                            start=(h == 0), stop=(h == NHL - 1))
                    if nj % 2 == 0:
                        nc.scalar.copy(osb[:, 512 * nj:512 * (nj + 1)], ops[:])
                    else:
                        nc.vector.tensor_copy(osb[:, 512 * nj:512 * (nj + 1)],
                                              ops[:])
                    if nj == 1:
                        nc.sync.dma_start(
                            out[128 * si:128 * (si + 1), 0:1024],
                            osb[:, 0:1024])
                nc.sync.dma_start(out[128 * si:128 * (si + 1), 1024:E],
                                  osb[:, 1024:E])

        def denom_norm(sp, h, rs, yps, yT):
            """Softmax denominator + normalize for head h of pass sp."""
            rsa = hot.tile([128, QC], F32, tag="rsa", bufs=2,
                           name=f"rsa{h}_{sp}")
            nc.gpsimd.partition_all_reduce(rsa[:], rs[:], 128,
                                           bass_isa.ReduceOp.add)
            rinv = hot.tile([128, QC], F32, tag="rinv", bufs=2,
                            name=f"rinv{h}_{sp}")
            nc.vector.reciprocal(rinv[:], rsa[:])
            nc.vector.tensor_mul(yT[:, QC * h:QC * (h + 1)], yps[:], rinv[:])

        def rope_swap(t_ap, name):
            """Issue the half-swap DMAs for rope; DVE math comes later."""
            tmp = stg.tile([128, PW], BF16, tag="ropetmp", bufs=5,
                           name=f"rt{name}")
            nc.sync.dma_start(tmp[0:64, :], t_ap[64:128, :])
            nc.sync.dma_start(tmp[64:128, :], t_ap[0:64, :])
            return tmp

        def rope_apply(tmp, t_ap, s0):
            """Buggy-faithful rope in place: t = (t + sgn*swap_half(t)) * sin."""
            nc.vector.scalar_tensor_tensor(tmp, tmp, sgn_sb, t_ap, MULT, ADD)
            nc.vector.tensor_mul(t_ap, tmp, sin_sb[:, s0:s0 + PW])

        def proj_group(sp, g, qT, xa, xb, ropes):
            """Projection-chain group g of pass sp: g0=k, g1=v(+transpose),
            g2=q0/q1, g3=q2/q3. Emitted as PE filler inside the previous
            pass's attention. Rope swap-DMAs issue here; the DVE math is
            deferred into `ropes` and applied at the end of the attention
            stretch."""
            s0 = PW * sp

            def xsl(e):
                t = xa if e < EC // 2 else xb
                return t[:, PW * (e % (EC // 2)):PW * (e % (EC // 2) + 1)]

            if g == 0:
                ps = psum.tile([128, PW], F32, tag="proj", bufs=2,
                               name=f"psk{sp}")
                for e in range(EC):
                    nc.tensor.matmul(ps[:], wk[:, D * e:D * (e + 1)], xsl(e),
                                     start=(e == 0), stop=(e == EC - 1))
                t_ap = kT_sb[:, s0:s0 + PW]
                nc.scalar.copy(t_ap, ps[:])
                ropes.append((rope_swap(t_ap, f"k{sp}")[:], t_ap, s0))
            elif g == 1:
                ps = psum.tile([128, PW], F32, tag="proj", bufs=2,
                               name=f"psv{sp}")
                for e in range(EC):
                    nc.tensor.matmul(ps[:], wv[:, D * e:D * (e + 1)], xsl(e),
                                     start=(e == 0), stop=(e == EC - 1))
                vstage = stg.tile([128, PW], BF16, tag="vstage", bufs=1,
                                  name=f"vst{sp}")
                nc.scalar.copy(vstage[:], ps[:])
                for j in range(PW // 128):
                    vt_ps = psum.tile([128, 128], BF16, tag="proj", bufs=2,
                                      name=f"vtr{sp}_{j}")
                    nc.tensor.transpose(vt_ps[:],
                                        vstage[:, 128 * j:128 * (j + 1)],
                                        ident_sb)
                    nc.scalar.copy(v_sb[s0 // 128 + j][:], vt_ps[:])
            else:
                for h in (2 * (g - 2), 2 * (g - 2) + 1):
                    ps = psum.tile([128, PW], F32, tag="proj", bufs=2,
                                   name=f"psq{sp}_{h}")
                    for e in range(EC):
                        nc.tensor.matmul(
                            ps[:],
                            wq[:, EC * D * h + D * e:EC * D * h + D * (e + 1)],
                            xsl(e), start=(e == 0), stop=(e == EC - 1))
                    t_ap = qT[:, PW * h:PW * (h + 1)]
                    nc.scalar.copy(t_ap, ps[:])
                    ropes.append((rope_swap(t_ap, f"q{sp}_{h}")[:], t_ap, s0))

        # ---- pass-0 projections upfront (startup); later passes are
        # emitted as PE filler inside the previous pass's attention
        yT_of = {}
        qT_cur = qpool.tile([128, NHL * PW], BF16, tag="qT", name="qT0")
        ropes0 = []
        for g in range(4):
            proj_group(0, g, qT_cur, xt0a, xt0b, ropes0)
        for args in ropes0:
            rope_apply(*args)

        xt_cur = (xt0a, xt0b)
        for sp in range(NSP):
            xa, xb = xt_cur
            qT = qT_cur
            ropes = []
            if sp + 1 < NSP:
                xt1a = xpool.tile([128, (EC // 2) * PW], BF16, tag="xt",
                                  name=f"xt{sp + 1}a")
                xt1b = xpool.tile([128, (EC // 2) * PW], BF16, tag="xt",
                                  name=f"xt{sp + 1}b")
                load_x_half(xt1a, sp + 1, 0)
                load_x_half(xt1b, sp + 1, 1)
                xt_cur = (xt1a, xt1b)
                qT_cur = qpool.tile([128, NHL * PW], BF16, tag="qT",
                                    name=f"qT{sp + 1}")

            # ---- causal attention for query chunk qj == sp
            yT = ypool.tile([128, NHL * QC], BF16, tag="yT", name=f"yT{sp}")
            yT_of[sp] = yT
            nki_hi = 4 * (sp + 1)
            prev = None  # deferred denom+normalize of the previous head
            for h in range(NHL):
                qh = qT[:, QC * h:QC * (h + 1)]
                yps = psum.tile([128, QC], F32, tag="yps", bufs=2,
                                name=f"yps{h}_{sp}")
                rs = hot.tile([128, QC], BF16, tag="rs", bufs=2,
                              name=f"rs{h}_{sp}")
                pend = []  # scores run a k-tile pair ahead of P@V
                pt0 = None
                for kp in range(0, nki_hi, 2):
                    # two k-tiles share a 2-bank PSUM tile and ONE exp op,
                    # halving the ACT access-latency overhead
                    st = psum.tile([128, 2 * QC], F32, tag="st", bufs=2,
                                   name=f"st{h}_{sp}_{kp}")
                    pt = hot.tile([128, 2 * QC], BF16, tag="pt", bufs=3,
                                  name=f"pt{h}_{sp}_{kp}")[:]
                    qlos = []
                    for j in (0, 1):
                        ki = kp + j
                        off = 128 * ki - QC * sp
                        qlo = max(0, off)  # fully-masked columns are skipped
                        qlos.append(qlo)
                        nc.tensor.matmul(
                            st[:, QC * j + qlo:QC * (j + 1)],
                            kT_sb[:, 128 * ki:128 * (ki + 1)],
                            qh[:, qlo:QC], start=True, stop=True)
                    # exp spans both slabs; the gap [QC:QC+qlo1) is stale
                    # PSUM whose exp lands in masked-out, never-read columns
                    nc.scalar.activation(pt[:, qlos[0]:2 * QC],
                                         st[:, qlos[0]:2 * QC], EXP,
                                         scale=scale)
                    for j in (0, 1):
                        ki = kp + j
                        off = 128 * ki - QC * sp
                        qlo = qlos[j]
                        ptj = pt[:, QC * j:QC * (j + 1)]
                        if off >= 0:
                            nc.vector.tensor_mul(ptj[:, qlo:qlo + 128],
                                                 ptj[:, qlo:qlo + 128],
                                                 mw_sb)
                        if ki == 0:
                            pt0 = ptj
                        elif ki == 1:
                            if qlo > 0:
                                nc.vector.tensor_copy(rs[:, 0:qlo],
                                                      pt0[:, 0:qlo])
                            nc.vector.tensor_add(rs[:, qlo:QC],
                                                 pt0[:, qlo:QC],
                                                 ptj[:, qlo:QC])
                        else:
                            nc.vector.tensor_add(rs[:, qlo:QC],
                                                 rs[:, qlo:QC],
                                                 ptj[:, qlo:QC])
                        pend.append((ki, qlo, ptj))
                    while len(pend) > 4:
                        k0, q0, p0 = pend.pop(0)
                        nc.tensor.matmul(yps[:, q0:QC], v_sb[k0][:],
                                         p0[:, q0:QC], start=(k0 == 0),
                                         stop=(k0 == nki_hi - 1))
                    if kp == 0 and prev is not None:
                        denom_norm(sp, *prev)
                        prev = None
                for k0, q0, p0 in pend:
                    nc.tensor.matmul(yps[:, q0:QC], v_sb[k0][:], p0[:, q0:QC],
                                     start=(k0 == 0), stop=(k0 == nki_hi - 1))
                prev = (h, rs, yps, yT)
                # PE filler between heads: prior pass's out-proj rows and the
                # next pass's projection chains
                if sp > 0:
                    out_proj_chunk(sp - 1, h)
                if sp + 1 < NSP:
                    proj_group(sp + 1, h, qT_cur, xt_cur[0], xt_cur[1], ropes)
            denom_norm(sp, *prev)
            for args in ropes:
                rope_apply(*args)

        for loc in range(QC // 128):
            out_proj_chunk(NSP - 1, loc)

    nc.compile()
    return nc


def make_consts(S, QC=512):
    """Host-precomputed constant tensors (rope table, causal mask, sign, identity)."""
    rope_dim = D // 2
    j = np.arange(rope_dim, dtype=np.float64)
    thetas = 1.0 / ROPE_BASE ** (2.0 * j / rope_dim)
    positions = np.arange(S, dtype=np.float64)
    angles = positions[:, None] * thetas[None, :]
    sin = np.sin(np.concatenate([angles, angles], axis=1)).astype(np.float32)  # [S, D]
    sinT = np.ascontiguousarray(sin.T)                                          # [D, S]
    sgn = np.where(np.arange(D) < rope_dim, -1.0, 1.0).astype(np.float32)

    B = QC - 128
    MW = 2 * QC - 128
    k_idx = np.arange(128)[:, None]
    c_idx = np.arange(MW)[None, :]
    mw = (k_idx <= (c_idx - B)).astype(np.float32)

    cst = np.concatenate([mw[:, B:B + 128], np.eye(128, dtype=np.float32),
                          sgn.reshape(128, 1)], axis=1)
    return {
        "sinT": np.ascontiguousarray(sinT.astype(NP_BF16)),
        "cst": np.ascontiguousarray(cst).astype(NP_BF16),
    }


def _swz_w(W):
    """[A*128, N] -> SBUF layout [128, A*N] (chunk-major along free dim)."""
    A = W.shape[0] // 128
    return np.ascontiguousarray(
        W.reshape(A, 128, W.shape[1]).transpose(1, 0, 2).reshape(128, -1)
    ).astype(NP_BF16)


def _swz_x(xb, S, E, PW):
    """[S, E] -> [128, NSP*2*(EC/2)*PW]: per pass-half, e-chunk-major tiles."""
    NSP, EC = S // PW, E // 128
    a = xb.reshape(NSP, PW, EC, 128).transpose(3, 0, 2, 1)   # [128,NSP,EC,PW]
    a = a.reshape(128, NSP, 2, (EC // 2) * PW)
    return np.ascontiguousarray(a.reshape(128, -1)).astype(NP_BF16)


def make_in_maps(x, Wq, Wk, Wv, Wo, S, E, QC=512, bo=None):
    """Shard full inputs into the 8 per-core input maps (bf16 on-device)."""
    consts = make_consts(S, QC)
    EC_ = E // 128
    in_maps = []
    for c in range(N_CORES):
        b, g = c // TP, c % TP
        m = dict(consts)
        m["xh"] = _swz_x(x[b], S, E, QC)
        wqg = Wq[:, NHL * D * g:NHL * D * (g + 1)]            # [E, NHL*D]
        wqg = wqg.reshape(EC_, 128, NHL, D).transpose(1, 2, 0, 3)  # [128,h,e,d]
        m["Wq"] = np.ascontiguousarray(
            wqg.reshape(128, -1)).astype(NP_BF16)
        m["Wk"] = _swz_w(Wk[:, D * g:D * (g + 1)])
        m["Wv"] = _swz_w(Wv[:, D * g:D * (g + 1)])
        m["Wo"] = _swz_w(Wo[NHL * D * g:NHL * D * (g + 1), :])
        in_maps.append(m)
    return in_maps


_CACHE = {}


def _compiled_full():
    if "nc" not in _CACHE:
        _CACHE["nc"] = build_program(S_FULL, E_FULL)
    return _CACHE["nc"]


def kernel(x, Wq, Wk, Wv, Wo, bo):
    nc = _compiled_full()
    in_maps = make_in_maps(x, Wq, Wk, Wv, Wo, S_FULL, E_FULL, bo=bo)
    res = run_bass_kernel_spmd(nc, in_maps, list(range(N_CORES)))
    # unshard the row-parallel out-proj: sum the 4 head-group partials,
    # then add the output bias (host side of the reduce)
    out = np.zeros((BATCH, S_FULL, E_FULL), np.float32)
    for c in range(N_CORES):
        out[c // TP] += res.results[c]["out"].astype(np.float32)
    out += bo.astype(np.float32)[None, None, :]
    return out


# revision 25
# speedup vs baseline: 1.0458x; 1.0284x over previous
"""Trainium2 Bass kernel: causal multi-head group attention (GQA) with RoPE.

Full-input contract: kernel(**inputs) takes the unsharded inputs and returns
the full output. Internally shards across 8 NeuronCores:
  core c -> (batch b = c // 4, head-group g = c % 4)
Each core computes 4 q heads + their single kv group end-to-end (QKV proj,
RoPE, causal flash-style attention, row-parallel out-proj partial). The host
unshard step sums the 4 head-group partials per batch and adds the output
bias.

The per-core program is a software pipeline over 512-token passes: each pass
projects q/k/v for its token window (bf16 operands, f32 PSUM accumulation),
applies RoPE, runs causal attention for that query chunk against all keys so
far, and defers the out-proj for those rows into the next pass so its PE work
never waits on the normalize chain. Matmul operands are bf16 (full PE rate at
any width); softmax numerator accumulation stays f32.

Engine placement keeps PE the only near-saturated engine:
  PE   - all matmuls, with attention scores emitted two k-tiles ahead of the
         P@V accumulation so PV's wait on exp never idles the PE queue
  ACT  - exp(softmax) + all PSUM->SBUF projection copies + half the out-proj
         staging copies
  DVE  - softmax numerator accumulation (f32), masking, reciprocal,
         y-normalize, other half of out-proj staging
  Pool - RoPE (fused (swap*sgn+t)*sin) + softmax-denominator partition
         all-reduce (keeps the PE queue free of ones-matmuls)
  DMA  - x/weight streaming (chunk-interleaved in consumption order), rope
         half-swaps, output writeback
"""

import os
import sys
from contextlib import ExitStack, nullcontext
from math import sqrt

for _p in ("/opt/trn_rl_repo", "/root/.axon_site/_ro/trn_rl_repo"):
    if os.path.isdir(_p) and _p not in sys.path:
        sys.path.insert(0, _p)

import numpy as np
import concourse.bacc as bacc
import concourse.tile as tile
import concourse.mybir as mybir
from concourse import bass_isa
from concourse.bass_utils import run_bass_kernel_spmd

F32 = mybir.dt.float32
BF16 = mybir.dt.bfloat16
NP_BF16 = mybir.dt.np(BF16)
EXP = mybir.ActivationFunctionType.Exp
MULT = mybir.AluOpType.mult
ADD = mybir.AluOpType.add

N_CORES = 8
TP = 4            # head-group parallel degree (within one batch element)
BATCH = 2
D = 128           # head dim
NHL = 4           # q heads per core
ROPE_BASE = 10000.0

# Full-problem config
S_FULL = 2048     # context length
E_FULL = 2048     # model dim


def build_program(S, E, QC=512, PW=512, n_cores=N_CORES, reps=1, staggered=False):
    """Emit the per-core SPMD program. QC: q-chunk width (attention moving dim),
    PW: phase-1 pass width over the sequence (must equal QC for the pipelined
    schedule). reps>1 wraps the whole body in an on-device For_i loop (timing
    builds only)."""
    assert PW == QC
    EC = E // 128     # contraction chunks over model dim
    NKI = S // 128    # k tiles
    NSP = S // PW     # passes
    B = QC - 128      # mask-table base offset
    MW = 2 * QC - 128 # mask-table width
    scale = 1.0 / sqrt(D)

    nc = bacc.Bacc("TRN2", target_bir_lowering=False, debug=False,
                   num_devices=n_cores)

    # host-preswizzled: every tensor is already in its SBUF tile layout so
    # each loads with ONE contiguous-line DMA
    xh = nc.dram_tensor("xh", [128, S * E // 128], BF16, kind="ExternalInput").ap()
    Wq = nc.dram_tensor("Wq", [128, EC * NHL * D], BF16, kind="ExternalInput").ap()
    Wk = nc.dram_tensor("Wk", [128, EC * D], BF16, kind="ExternalInput").ap()
    Wv = nc.dram_tensor("Wv", [128, EC * D], BF16, kind="ExternalInput").ap()
    Wo = nc.dram_tensor("Wo", [128, NHL * E], BF16, kind="ExternalInput").ap()
    sinT = nc.dram_tensor("sinT", [D, S], BF16, kind="ExternalInput").ap()
    cst = nc.dram_tensor("cst", [128, 257], BF16, kind="ExternalInput").ap()
    out = nc.dram_tensor("out", [S, E], BF16, kind="ExternalOutput").ap()

    with tile.TileContext(nc) as tc, \
         (tc.For_i(0, reps, 1, staggered_reset=staggered) if reps > 1 else nullcontext()), \
         nc.allow_low_precision(reason="bf16 internal compute, f32 psum"), \
         ExitStack() as top:
        pers = top.enter_context(tc.tile_pool(name="pers", bufs=1))
        kT_sb = pers.tile([128, S], BF16, name="kT_sb")
        v_sb = [pers.tile([128, D], BF16, tag=f"v{i}", name=f"v{i}")
                for i in range(NKI)]
        wq = pers.tile([128, EC * NHL * D], BF16, tag="wq", name="wq")
        wk = pers.tile([128, EC * D], BF16, tag="wk", name="wk")
        wv = pers.tile([128, EC * D], BF16, tag="wv", name="wv")
        wo = pers.tile([128, NHL * E], BF16, tag="wo", name="wo")
        sin_sb = pers.tile([128, S], BF16, tag="sin", name="sin_sb")
        cst_sb = pers.tile([128, 257], BF16, tag="cst", name="cst_sb")
        mw_sb = cst_sb[:, 0:128]
        ident_sb = cst_sb[:, 128:256]
        sgn_sb = cst_sb[:, 256:257]

        # PSUM pool: per-tag buffer counts sum to exactly 8 banks.
        psum = top.enter_context(tc.tile_pool(name="psum", bufs=1, space="PSUM"))
        # Rotating SBUF pools.
        xpool = top.enter_context(tc.tile_pool(name="xt", bufs=4))
        qpool = top.enter_context(tc.tile_pool(name="qp", bufs=2))
        ypool = top.enter_context(tc.tile_pool(name="yp", bufs=2))
        opool = top.enter_context(tc.tile_pool(name="osb", bufs=2))
        stg = top.enter_context(tc.tile_pool(name="stg", bufs=2))
        hot = top.enter_context(tc.tile_pool(name="hot", bufs=1))

        # ---- startup DMAs: small consts first, then chunk-interleaved
        # weight / x streaming in roughly the order the PE consumes them.
        nc.sync.dma_start(cst_sb[:], cst[:])

        HB = (EC // 2) * PW  # elements per x half-tile

        def load_x_half(xt_tile, sp, half):
            off = (sp * 2 + half) * HB
            nc.sync.dma_start(xt_tile[:], xh[:, off:off + HB])

        # pass-0 x and weights, ordered so the first projection chains can
        # start as soon as their operands land
        xt0a = xpool.tile([128, HB], BF16, tag="xt", bufs=4, name="xt0a")
        xt0b = xpool.tile([128, HB], BF16, tag="xt", name="xt0b")
        WQH = EC * D  # one head's worth of wq columns (head-major layout)
        # every projection chain contracts over the FULL x pass, so x must
        # finish streaming before ANY chain completes: x goes first, then
        # the weights in chain-consumption order
        nc.sync.dma_start(wk[:], Wk[:])
        nc.sync.dma_start(xt0a[:, 0:HB // 2], xh[:, 0:HB // 2])
        nc.sync.dma_start(xt0a[:, HB // 2:HB], xh[:, HB // 2:HB])
        nc.sync.dma_start(xt0b[:, 0:HB // 2], xh[:, HB:HB + HB // 2])
        nc.sync.dma_start(xt0b[:, HB // 2:HB], xh[:, HB + HB // 2:2 * HB])
        nc.sync.dma_start(wv[:], Wv[:])
        for hh in range(NHL):
            nc.sync.dma_start(wq[:, hh * WQH:(hh + 1) * WQH],
                              Wq[:, hh * WQH:(hh + 1) * WQH])
        nc.sync.dma_start(sin_sb[:], sinT[:])
        nc.sync.dma_start(wo[:], Wo[:])

        def out_proj_chunk(sp, loc):
            """Row-parallel Wo partial for row-block loc of pass sp."""
            yT = yT_of[sp]
            if True:
                si = (QC // 128) * sp + loc
                osb = opool.tile([128, E], BF16, tag="osb", name=f"osb{si}")
                for nj in range(E // 512):
                    ops = psum.tile([128, 512], F32, tag="proj", bufs=2,
                                    name=f"ops{si}_{nj}")
                    for h in range(NHL):
                        nc.tensor.matmul(
                            ops[:],
                            yT[:, QC * h + 128 * loc:QC * h + 128 * (loc + 1)],
                            wo[:, E * h + 512 * nj:E * h + 512 * (nj + 1)],
                            start=(h == 0), stop=(h == NHL - 1))
                    if nj % 2 == 0:
                        nc.scalar.copy(osb[:, 512 * nj:512 * (nj + 1)], ops[:])
                    else:
                        nc.vector.tensor_copy(osb[:, 512 * nj:512 * (nj + 1)],
                                              ops[:])
                    if nj == 1:
                        nc.sync.dma_start(
                            out[128 * si:128 * (si + 1), 0:1024],
                            osb[:, 0:1024])
                nc.sync.dma_start(out[128 * si:128 * (si + 1), 1024:E],
                                  osb[:, 1024:E])

        def denom_norm(sp, h, rs, yps, yT):
            """Softmax denominator + normalize for head h of pass sp."""
            rsa = hot.tile([128, QC], F32, tag="rsa", bufs=2,
                           name=f"rsa{h}_{sp}")
            nc.gpsimd.partition_all_reduce(rsa[:], rs[:], 128,
                                           bass_isa.ReduceOp.add)
            rinv = hot.tile([128, QC], F32, tag="rinv", bufs=2,
                            name=f"rinv{h}_{sp}")
            nc.vector.reciprocal(rinv[:], rsa[:])
            nc.vector.tensor_mul(yT[:, QC * h:QC * (h + 1)], yps[:], rinv[:])

        def rope_swap(t_ap, name):
            """Issue the half-swap DMAs for rope; DVE math comes later."""
            tmp = stg.tile([128, PW], BF16, tag="ropetmp", bufs=5,
                           name=f"rt{name}")
            nc.sync.dma_start(tmp[0:64, :], t_ap[64:128, :])
            nc.sync.dma_start(tmp[64:128, :], t_ap[0:64, :])
            return tmp

        def rope_apply(tmp, t_ap, s0):
            """Buggy-faithful rope in place: t = (t + sgn*swap_half(t)) * sin."""
            nc.vector.scalar_tensor_tensor(tmp, tmp, sgn_sb, t_ap, MULT, ADD)
            nc.vector.tensor_mul(t_ap, tmp, sin_sb[:, s0:s0 + PW])

        def proj_group(sp, g, qT, xa, xb, ropes):
            """Projection-chain group g of pass sp: g0=k, g1=v(+transpose),
            g2=q0/q1, g3=q2/q3. Emitted as PE filler inside the previous
            pass's attention. Rope swap-DMAs issue here; the DVE math is
            deferred into `ropes` and applied at the end of the attention
            stretch."""
            s0 = PW * sp

            def xsl(e):
                t = xa if e < EC // 2 else xb
                return t[:, PW * (e % (EC // 2)):PW * (e % (EC // 2) + 1)]

            if g == 0:
                ps = psum.tile([128, PW], F32, tag="proj", bufs=2,
                               name=f"psk{sp}")
                for e in range(EC):
                    nc.tensor.matmul(ps[:], wk[:, D * e:D * (e + 1)], xsl(e),
                                     start=(e == 0), stop=(e == EC - 1))
                t_ap = kT_sb[:, s0:s0 + PW]
                nc.scalar.copy(t_ap, ps[:])
                ropes.append((rope_swap(t_ap, f"k{sp}")[:], t_ap, s0))
            elif g == 1:
                ps = psum.tile([128, PW], F32, tag="proj", bufs=2,
                               name=f"psv{sp}")
                for e in range(EC):
                    nc.tensor.matmul(ps[:], wv[:, D * e:D * (e + 1)], xsl(e),
                                     start=(e == 0), stop=(e == EC - 1))
                vstage = stg.tile([128, PW], BF16, tag="vstage", bufs=1,
                                  name=f"vst{sp}")
                nc.scalar.copy(vstage[:], ps[:])
                for j in range(PW // 128):
                    vt_ps = psum.tile([128, 128], BF16, tag="proj", bufs=2,
                                      name=f"vtr{sp}_{j}")
                    nc.tensor.transpose(vt_ps[:],
                                        vstage[:, 128 * j:128 * (j + 1)],
                                        ident_sb)
                    nc.scalar.copy(v_sb[s0 // 128 + j][:], vt_ps[:])
            else:
                for h in (2 * (g - 2), 2 * (g - 2) + 1):
                    ps = psum.tile([128, PW], F32, tag="proj", bufs=2,
                                   name=f"psq{sp}_{h}")
                    for e in range(EC):
                        nc.tensor.matmul(
                            ps[:],
                            wq[:, EC * D * h + D * e:EC * D * h + D * (e + 1)],
                            xsl(e), start=(e == 0), stop=(e == EC - 1))
                    t_ap = qT[:, PW * h:PW * (h + 1)]
                    nc.scalar.copy(t_ap, ps[:])
                    ropes.append((rope_swap(t_ap, f"q{sp}_{h}")[:], t_ap, s0))

        # ---- pass-0 projections upfront (startup); later passes are
        # emitted as PE filler inside the previous pass's attention
        yT_of = {}
        qT_cur = qpool.tile([128, NHL * PW], BF16, tag="qT", name="qT0")
        ropes0 = []
        for g in range(4):
            proj_group(0, g, qT_cur, xt0a, xt0b, ropes0)
        for args in ropes0:
            rope_apply(*args)

        xt_cur = (xt0a, xt0b)
        for sp in range(NSP):
            xa, xb = xt_cur
            qT = qT_cur
            ropes = []
            if sp + 1 < NSP:
                xt1a = xpool.tile([128, (EC // 2) * PW], BF16, tag="xt",
                                  name=f"xt{sp + 1}a")
                xt1b = xpool.tile([128, (EC // 2) * PW], BF16, tag="xt",
                                  name=f"xt{sp + 1}b")
                load_x_half(xt1a, sp + 1, 0)
                load_x_half(xt1b, sp + 1, 1)
                xt_cur = (xt1a, xt1b)
                qT_cur = qpool.tile([128, NHL * PW], BF16, tag="qT",
                                    name=f"qT{sp + 1}")

            # ---- causal attention for query chunk qj == sp
            yT = ypool.tile([128, NHL * QC], BF16, tag="yT", name=f"yT{sp}")
            yT_of[sp] = yT
            nki_hi = 4 * (sp + 1)
            prev = None  # deferred denom+normalize of the previous head
            for h in range(NHL):
                qh = qT[:, QC * h:QC * (h + 1)]
                yps = psum.tile([128, QC], F32, tag="yps", bufs=2,
                                name=f"yps{h}_{sp}")
                rs = hot.tile([128, QC], BF16, tag="rs", bufs=2,
                              name=f"rs{h}_{sp}")
                pend = []  # scores run a k-tile pair ahead of P@V
                pt0 = None
                for kp in range(0, nki_hi, 2):
                    # two k-tiles share a 2-bank PSUM tile and ONE exp op,
                    # halving the ACT access-latency overhead
                    st = psum.tile([128, 2 * QC], F32, tag="st", bufs=2,
                                   name=f"st{h}_{sp}_{kp}")
                    pt = hot.tile([128, 2 * QC], BF16, tag="pt", bufs=3,
                                  name=f"pt{h}_{sp}_{kp}")[:]
                    qlos = []
                    for j in (0, 1):
                        ki = kp + j
                        off = 128 * ki - QC * sp
                        qlo = max(0, off)  # fully-masked columns are skipped
                        qlos.append(qlo)
                        nc.tensor.matmul(
                            st[:, QC * j + qlo:QC * (j + 1)],
                            kT_sb[:, 128 * ki:128 * (ki + 1)],
                            qh[:, qlo:QC], start=True, stop=True)
                    # exp spans both slabs; the gap [QC:QC+qlo1) is stale
                    # PSUM whose exp lands in masked-out, never-read columns
                    nc.scalar.activation(pt[:, qlos[0]:2 * QC],
                                         st[:, qlos[0]:2 * QC], EXP,
                                         scale=scale)
                    for j in (0, 1):
                        ki = kp + j
                        off = 128 * ki - QC * sp
                        qlo = qlos[j]
                        ptj = pt[:, QC * j:QC * (j + 1)]
                        if off >= 0:
                            nc.vector.tensor_mul(ptj[:, qlo:qlo + 128],
                                                 ptj[:, qlo:qlo + 128],
                                                 mw_sb)
                        if ki == 0:
                            pt0 = ptj
                        elif ki == 1:
                            if qlo > 0:
                                nc.vector.tensor_copy(rs[:, 0:qlo],
                                                      pt0[:, 0:qlo])
                            nc.vector.tensor_add(rs[:, qlo:QC],
                                                 pt0[:, qlo:QC],
                                                 ptj[:, qlo:QC])
                        else:
                            nc.vector.tensor_add(rs[:, qlo:QC],
                                                 rs[:, qlo:QC],
                                                 ptj[:, qlo:QC])
                        pend.append((ki, qlo, ptj))
                    while len(pend) > 4:
                        k0, q0, p0 = pend.pop(0)
                        nc.tensor.matmul(yps[:, q0:QC], v_sb[k0][:],
                                         p0[:, q0:QC], start=(k0 == 0),
                                         stop=(k0 == nki_hi - 1))
                    if kp == 0 and prev is not None:
                        denom_norm(sp, *prev)
                        prev = None
                for k0, q0, p0 in pend:
                    nc.tensor.matmul(yps[:, q0:QC], v_sb[k0][:], p0[:, q0:QC],
                                     start=(k0 == 0), stop=(k0 == nki_hi - 1))
                prev = (h, rs, yps, yT)
                # PE filler between heads: prior pass's out-proj rows and the
                # next pass's projection chains
                if sp > 0:
                    out_proj_chunk(sp - 1, h)
                if sp + 1 < NSP:
                    proj_group(sp + 1, h, qT_cur, xt_cur[0], xt_cur[1], ropes)
            denom_norm(sp, *prev)
            for args in ropes:
                rope_apply(*args)

        for loc in range(QC // 128):
            out_proj_chunk(NSP - 1, loc)

    nc.compile()
    return nc


def make_consts(S, QC=512):
    """Host-precomputed constant tensors (rope table, causal mask, sign, identity)."""
    rope_dim = D // 2
    j = np.arange(rope_dim, dtype=np.float64)
    thetas = 1.0 / ROPE_BASE ** (2.0 * j / rope_dim)
    positions = np.arange(S, dtype=np.float64)
    angles = positions[:, None] * thetas[None, :]
    sin = np.sin(np.concatenate([angles, angles], axis=1)).astype(np.float32)  # [S, D]
    sinT = np.ascontiguousarray(sin.T)                                          # [D, S]
    sgn = np.where(np.arange(D) < rope_dim, -1.0, 1.0).astype(np.float32)

    B = QC - 128
    MW = 2 * QC - 128
    k_idx = np.arange(128)[:, None]
    c_idx = np.arange(MW)[None, :]
    mw = (k_idx <= (c_idx - B)).astype(np.float32)

    cst = np.concatenate([mw[:, B:B + 128], np.eye(128, dtype=np.float32),
                          sgn.reshape(128, 1)], axis=1)
    return {
        "sinT": np.ascontiguousarray(sinT.astype(NP_BF16)),
        "cst": np.ascontiguousarray(cst).astype(NP_BF16),
    }


def _swz_w(W):
    """[A*128, N] -> SBUF layout [128, A*N] (chunk-major along free dim)."""
    A = W.shape[0] // 128
    return np.ascontiguousarray(
        W.reshape(A, 128, W.shape[1]).transpose(1, 0, 2).reshape(128, -1)
    ).astype(NP_BF16)


def _swz_x(xb, S, E, PW):
    """[S, E] -> [128, NSP*2*(EC/2)*PW]: per pass-half, e-chunk-major tiles."""
    NSP, EC = S // PW, E // 128
    a = xb.reshape(NSP, PW, EC, 128).transpose(3, 0, 2, 1)   # [128,NSP,EC,PW]
    a = a.reshape(128, NSP, 2, (EC // 2) * PW)
    return np.ascontiguousarray(a.reshape(128, -1)).astype(NP_BF16)


def make_in_maps(x, Wq, Wk, Wv, Wo, S, E, QC=512, bo=None):
    """Shard full inputs into the 8 per-core input maps (bf16 on-device)."""
    consts = make_consts(S, QC)
    EC_ = E // 128
    in_maps = []
    for c in range(N_CORES):
        b, g = c // TP, c % TP
        m = dict(consts)
        m["xh"] = _swz_x(x[b], S, E, QC)
        wqg = Wq[:, NHL * D * g:NHL * D * (g + 1)]            # [E, NHL*D]
        wqg = wqg.reshape(EC_, 128, NHL, D).transpose(1, 2, 0, 3)  # [128,h,e,d]
        m["Wq"] = np.ascontiguousarray(
            wqg.reshape(128, -1)).astype(NP_BF16)
        m["Wk"] = _swz_w(Wk[:, D * g:D * (g + 1)])
        m["Wv"] = _swz_w(Wv[:, D * g:D * (g + 1)])
        m["Wo"] = _swz_w(Wo[NHL * D * g:NHL * D * (g + 1), :])
        in_maps.append(m)
    return in_maps


_CACHE = {}


def _compiled_full():
    if "nc" not in _CACHE:
        _CACHE["nc"] = build_program(S_FULL, E_FULL)
    return _CACHE["nc"]


def kernel(x, Wq, Wk, Wv, Wo, bo):
    nc = _compiled_full()
    in_maps = make_in_maps(x, Wq, Wk, Wv, Wo, S_FULL, E_FULL, bo=bo)
    res = run_bass_kernel_spmd(nc, in_maps, list(range(N_CORES)))
    # unshard the row-parallel out-proj: sum the 4 head-group partials,
    # then add the output bias (host side of the reduce)
    out = np.zeros((BATCH, S_FULL, E_FULL), np.float32)
    for c in range(N_CORES):
        out[c // TP] += res.results[c]["out"].astype(np.float32)
    out += bo.astype(np.float32)[None, None, :]
    return out


# revision 27
# speedup vs baseline: 1.0994x; 1.0512x over previous
"""Trainium2 Bass kernel: causal multi-head group attention (GQA) with RoPE.

Full-input contract: kernel(**inputs) takes the unsharded inputs and returns
the full output. Internally shards across 8 NeuronCores:
  core c -> (batch b = c // 4, head-group g = c % 4)
Each core computes 4 q heads + their single kv group end-to-end (QKV proj,
RoPE, causal flash-style attention, row-parallel out-proj partial). The host
unshard step sums the 4 head-group partials per batch and adds the output
bias.

The per-core program is a software pipeline over 512-token passes: each pass
projects q/k/v for its token window (bf16 operands, f32 PSUM accumulation),
applies RoPE, runs causal attention for that query chunk against all keys so
far, and defers the out-proj for those rows into the next pass so its PE work
never waits on the normalize chain. Matmul operands are bf16 (full PE rate at
any width); softmax numerator accumulation stays f32.

Engine placement keeps PE the only near-saturated engine:
  PE   - all matmuls, with attention scores emitted two k-tiles ahead of the
         P@V accumulation so PV's wait on exp never idles the PE queue
  ACT  - exp(softmax) + all PSUM->SBUF projection copies + half the out-proj
         staging copies
  DVE  - softmax numerator accumulation (f32), masking, reciprocal,
         y-normalize, other half of out-proj staging
  Pool - RoPE (fused (swap*sgn+t)*sin) + softmax-denominator partition
         all-reduce (keeps the PE queue free of ones-matmuls)
  DMA  - x/weight streaming (chunk-interleaved in consumption order), rope
         half-swaps, output writeback
"""

import os
import sys
from contextlib import ExitStack, nullcontext
from math import sqrt

for _p in ("/opt/trn_rl_repo", "/root/.axon_site/_ro/trn_rl_repo"):
    if os.path.isdir(_p) and _p not in sys.path:
        sys.path.insert(0, _p)

import numpy as np
import concourse.bacc as bacc
import concourse.tile as tile
import concourse.mybir as mybir
from concourse import bass_isa
from concourse.bass_utils import run_bass_kernel_spmd

F32 = mybir.dt.float32
BF16 = mybir.dt.bfloat16
NP_BF16 = mybir.dt.np(BF16)
EXP = mybir.ActivationFunctionType.Exp
MULT = mybir.AluOpType.mult
ADD = mybir.AluOpType.add

N_CORES = 8
TP = 4            # head-group parallel degree (within one batch element)
BATCH = 2
D = 128           # head dim
NHL = 4           # q heads per core
ROPE_BASE = 10000.0

# Full-problem config
S_FULL = 2048     # context length
E_FULL = 2048     # model dim


def build_program(S, E, QC=512, PW=512, n_cores=N_CORES, reps=1, staggered=False):
    """Emit the per-core SPMD program. QC: q-chunk width (attention moving dim),
    PW: phase-1 pass width over the sequence (must equal QC for the pipelined
    schedule). reps>1 wraps the whole body in an on-device For_i loop (timing
    builds only)."""
    assert PW == QC
    EC = E // 128     # contraction chunks over model dim
    NKI = S // 128    # k tiles
    NSP = S // PW     # passes
    B = QC - 128      # mask-table base offset
    MW = 2 * QC - 128 # mask-table width
    scale = 1.0 / sqrt(D)

    nc = bacc.Bacc("TRN2", target_bir_lowering=False, debug=False,
                   num_devices=n_cores)

    # host-preswizzled: every tensor is already in its SBUF tile layout so
    # each loads with ONE contiguous-line DMA
    xh = nc.dram_tensor("xh", [128, S * E // 128], BF16, kind="ExternalInput").ap()
    Wq = nc.dram_tensor("Wq", [128, EC * NHL * D], BF16, kind="ExternalInput").ap()
    Wk = nc.dram_tensor("Wk", [128, EC * D], BF16, kind="ExternalInput").ap()
    Wv = nc.dram_tensor("Wv", [128, EC * D], BF16, kind="ExternalInput").ap()
    Wo = nc.dram_tensor("Wo", [128, NHL * E], BF16, kind="ExternalInput").ap()
    sinT = nc.dram_tensor("sinT", [D, S], BF16, kind="ExternalInput").ap()
    cst = nc.dram_tensor("cst", [128, 257], BF16, kind="ExternalInput").ap()
    out = nc.dram_tensor("out", [S, E], BF16, kind="ExternalOutput").ap()

    with tile.TileContext(nc) as tc, \
         (tc.For_i(0, reps, 1, staggered_reset=staggered) if reps > 1 else nullcontext()), \
         nc.allow_low_precision(reason="bf16 internal compute, f32 psum"), \
         ExitStack() as top:
        pers = top.enter_context(tc.tile_pool(name="pers", bufs=1))
        kT_sb = pers.tile([128, S], BF16, name="kT_sb")
        v_sb = [pers.tile([128, D], BF16, tag=f"v{i}", name=f"v{i}")
                for i in range(NKI)]
        wq = pers.tile([128, EC * NHL * D], BF16, tag="wq", name="wq")
        wk = pers.tile([128, EC * D], BF16, tag="wk", name="wk")
        wv = pers.tile([128, EC * D], BF16, tag="wv", name="wv")
        wo = pers.tile([128, NHL * E], BF16, tag="wo", name="wo")
        sin_sb = pers.tile([128, S], BF16, tag="sin", name="sin_sb")
        cst_sb = pers.tile([128, 257], BF16, tag="cst", name="cst_sb")
        mw_sb = cst_sb[:, 0:128]
        ident_sb = cst_sb[:, 128:256]
        sgn_sb = cst_sb[:, 256:257]

        # PSUM pool: per-tag buffer counts sum to exactly 8 banks.
        psum = top.enter_context(tc.tile_pool(name="psum", bufs=1, space="PSUM"))
        # Rotating SBUF pools.
        xpool = top.enter_context(tc.tile_pool(name="xt", bufs=4))
        qpool = top.enter_context(tc.tile_pool(name="qp", bufs=2))
        ypool = top.enter_context(tc.tile_pool(name="yp", bufs=2))
        opool = top.enter_context(tc.tile_pool(name="osb", bufs=2))
        stg = top.enter_context(tc.tile_pool(name="stg", bufs=2))
        hot = top.enter_context(tc.tile_pool(name="hot", bufs=1))

        # ---- startup DMAs: small consts first, then chunk-interleaved
        # weight / x streaming in roughly the order the PE consumes them.
        nc.sync.dma_start(cst_sb[:], cst[:])

        HB = (EC // 2) * PW  # elements per x half-tile

        def load_x_half(xt_tile, sp, half):
            off = (sp * 2 + half) * HB
            nc.sync.dma_start(xt_tile[:], xh[:, off:off + HB])

        # pass-0 x and weights, ordered so the first projection chains can
        # start as soon as their operands land
        xt0a = xpool.tile([128, HB], BF16, tag="xt", bufs=4, name="xt0a")
        xt0b = xpool.tile([128, HB], BF16, tag="xt", name="xt0b")
        WQH = EC * D  # one head's worth of wq columns (head-major layout)
        # every projection chain contracts over the FULL x pass, so x must
        # finish streaming before ANY chain completes: x goes first, then
        # the weights in chain-consumption order
        nc.sync.dma_start(wk[:], Wk[:])
        nc.sync.dma_start(xt0a[:, 0:HB // 2], xh[:, 0:HB // 2])
        nc.sync.dma_start(xt0a[:, HB // 2:HB], xh[:, HB // 2:HB])
        nc.sync.dma_start(xt0b[:, 0:HB // 2], xh[:, HB:HB + HB // 2])
        nc.sync.dma_start(xt0b[:, HB // 2:HB], xh[:, HB + HB // 2:2 * HB])
        nc.sync.dma_start(wv[:], Wv[:])
        for hh in range(NHL):
            nc.sync.dma_start(wq[:, hh * WQH:(hh + 1) * WQH],
                              Wq[:, hh * WQH:(hh + 1) * WQH])
        nc.sync.dma_start(sin_sb[:], sinT[:])
        nc.sync.dma_start(wo[:], Wo[:])

        def out_proj_chunk(sp, loc):
            """Row-parallel Wo partial for row-block loc of pass sp."""
            yT = yT_of[sp]
            if True:
                si = (QC // 128) * sp + loc
                osb = opool.tile([128, E], BF16, tag="osb", name=f"osb{si}")
                for nj in range(E // 512):
                    ops = psum.tile([128, 512], F32, tag="proj", bufs=2,
                                    name=f"ops{si}_{nj}")
                    for h in range(NHL):
                        nc.tensor.matmul(
                            ops[:],
                            yT[:, QC * h + 128 * loc:QC * h + 128 * (loc + 1)],
                            wo[:, E * h + 512 * nj:E * h + 512 * (nj + 1)],
                            start=(h == 0), stop=(h == NHL - 1))
                    if nj % 2 == 0:
                        nc.scalar.copy(osb[:, 512 * nj:512 * (nj + 1)], ops[:])
                    else:
                        nc.vector.tensor_copy(osb[:, 512 * nj:512 * (nj + 1)],
                                              ops[:])
                    if nj == 1:
                        nc.sync.dma_start(
                            out[128 * si:128 * (si + 1), 0:1024],
                            osb[:, 0:1024])
                nc.sync.dma_start(out[128 * si:128 * (si + 1), 1024:E],
                                  osb[:, 1024:E])

        def denom_norm(sp, h, rs, yps, yT):
            """Softmax denominator + normalize for head h of pass sp."""
            rsa = hot.tile([128, QC], F32, tag="rsa", bufs=2,
                           name=f"rsa{h}_{sp}")
            nc.gpsimd.partition_all_reduce(rsa[:], rs[:], 128,
                                           bass_isa.ReduceOp.add)
            rinv = hot.tile([128, QC], F32, tag="rinv", bufs=2,
                            name=f"rinv{h}_{sp}")
            nc.vector.reciprocal(rinv[:], rsa[:])
            nc.vector.tensor_mul(yT[:, QC * h:QC * (h + 1)], yps[:], rinv[:])

        def rope_swap(t_ap, name):
            """Issue the half-swap DMAs for rope; DVE math comes later."""
            tmp = stg.tile([128, PW], BF16, tag="ropetmp", bufs=5,
                           name=f"rt{name}")
            nc.sync.dma_start(tmp[0:64, :], t_ap[64:128, :])
            nc.sync.dma_start(tmp[64:128, :], t_ap[0:64, :])
            return tmp

        def rope_apply(tmp, t_ap, s0):
            """Buggy-faithful rope in place: t = (t + sgn*swap_half(t)) * sin."""
            nc.vector.scalar_tensor_tensor(tmp, tmp, sgn_sb, t_ap, MULT, ADD)
            nc.vector.tensor_mul(t_ap, tmp, sin_sb[:, s0:s0 + PW])

        def proj_group(sp, g, qT, xa, xb, ropes):
            """Projection-chain group g of pass sp: g0=k, g1=v(+transpose),
            g2=q0/q1, g3=q2/q3. Emitted as PE filler inside the previous
            pass's attention. Rope swap-DMAs issue here; the DVE math is
            deferred into `ropes` and applied at the end of the attention
            stretch."""
            s0 = PW * sp

            def xsl(e):
                t = xa if e < EC // 2 else xb
                return t[:, PW * (e % (EC // 2)):PW * (e % (EC // 2) + 1)]

            if g == 0:
                ps = psum.tile([128, PW], F32, tag="proj", bufs=2,
                               name=f"psk{sp}")
                for e in range(EC):
                    nc.tensor.matmul(ps[:], wk[:, D * e:D * (e + 1)], xsl(e),
                                     start=(e == 0), stop=(e == EC - 1))
                t_ap = kT_sb[:, s0:s0 + PW]
                nc.scalar.copy(t_ap, ps[:])
                ropes.append((rope_swap(t_ap, f"k{sp}")[:], t_ap, s0))
            elif g == 1:
                ps = psum.tile([128, PW], F32, tag="proj", bufs=2,
                               name=f"psv{sp}")
                for e in range(EC):
                    nc.tensor.matmul(ps[:], wv[:, D * e:D * (e + 1)], xsl(e),
                                     start=(e == 0), stop=(e == EC - 1))
                vstage = stg.tile([128, PW], BF16, tag="vstage", bufs=1,
                                  name=f"vst{sp}")
                nc.scalar.copy(vstage[:], ps[:])
                for j in range(PW // 128):
                    vt_ps = psum.tile([128, 128], BF16, tag="proj", bufs=2,
                                      name=f"vtr{sp}_{j}")
                    nc.tensor.transpose(vt_ps[:],
                                        vstage[:, 128 * j:128 * (j + 1)],
                                        ident_sb)
                    nc.scalar.copy(v_sb[s0 // 128 + j][:], vt_ps[:])
            else:
                for h in (2 * (g - 2), 2 * (g - 2) + 1):
                    ps = psum.tile([128, PW], F32, tag="proj", bufs=2,
                                   name=f"psq{sp}_{h}")
                    for e in range(EC):
                        nc.tensor.matmul(
                            ps[:],
                            wq[:, EC * D * h + D * e:EC * D * h + D * (e + 1)],
                            xsl(e), start=(e == 0), stop=(e == EC - 1))
                    t_ap = qT[:, PW * h:PW * (h + 1)]
                    nc.scalar.copy(t_ap, ps[:])
                    ropes.append((rope_swap(t_ap, f"q{sp}_{h}")[:], t_ap, s0))

        # ---- pass-0 projections upfront (startup); later passes are
        # emitted as PE filler inside the previous pass's attention
        yT_of = {}
        qT_cur = qpool.tile([128, NHL * PW], BF16, tag="qT", name="qT0")
        ropes0 = []
        for g in range(4):
            proj_group(0, g, qT_cur, xt0a, xt0b, ropes0)
        for args in ropes0:
            rope_apply(*args)

        xt_cur = (xt0a, xt0b)
        for sp in range(NSP):
            xa, xb = xt_cur
            qT = qT_cur
            ropes = []
            if sp + 1 < NSP:
                xt1a = xpool.tile([128, (EC // 2) * PW], BF16, tag="xt",
                                  name=f"xt{sp + 1}a")
                xt1b = xpool.tile([128, (EC // 2) * PW], BF16, tag="xt",
                                  name=f"xt{sp + 1}b")
                load_x_half(xt1a, sp + 1, 0)
                load_x_half(xt1b, sp + 1, 1)
                xt_cur = (xt1a, xt1b)
                qT_cur = qpool.tile([128, NHL * PW], BF16, tag="qT",
                                    name=f"qT{sp + 1}")

            # ---- causal attention for query chunk qj == sp
            yT = ypool.tile([128, NHL * QC], BF16, tag="yT", name=f"yT{sp}")
            yT_of[sp] = yT
            nki_hi = 4 * (sp + 1)
            prev = None  # deferred denom+normalize of the previous head
            for h in range(NHL):
                qh = qT[:, QC * h:QC * (h + 1)]
                yps = psum.tile([128, QC], F32, tag="yps", bufs=2,
                                name=f"yps{h}_{sp}")
                rs = hot.tile([128, QC], BF16, tag="rs", bufs=2,
                              name=f"rs{h}_{sp}")
                pend = []  # scores run a k-tile pair ahead of P@V
                pt0 = None
                for kp in range(0, nki_hi, 2):
                    # two k-tiles share a 2-bank PSUM tile and ONE exp op,
                    # halving the ACT access-latency overhead
                    st = psum.tile([128, 2 * QC], F32, tag="st", bufs=2,
                                   name=f"st{h}_{sp}_{kp}")
                    pt = hot.tile([128, 2 * QC], BF16, tag="pt", bufs=3,
                                  name=f"pt{h}_{sp}_{kp}")[:]
                    qlos = []
                    for j in (0, 1):
                        ki = kp + j
                        off = 128 * ki - QC * sp
                        qlo = max(0, off)  # fully-masked columns are skipped
                        qlos.append(qlo)
                        nc.tensor.matmul(
                            st[:, QC * j + qlo:QC * (j + 1)],
                            kT_sb[:, 128 * ki:128 * (ki + 1)],
                            qh[:, qlo:QC], start=True, stop=True)
                    # exp spans both slabs; the gap [QC:QC+qlo1) is stale
                    # PSUM whose exp lands in masked-out, never-read columns
                    nc.scalar.activation(pt[:, qlos[0]:2 * QC],
                                         st[:, qlos[0]:2 * QC], EXP,
                                         scale=scale)
                    for j in (0, 1):
                        ki = kp + j
                        off = 128 * ki - QC * sp
                        qlo = qlos[j]
                        ptj = pt[:, QC * j:QC * (j + 1)]
                        if off >= 0:
                            nc.vector.tensor_mul(ptj[:, qlo:qlo + 128],
                                                 ptj[:, qlo:qlo + 128],
                                                 mw_sb)
                        if ki == 0:
                            pt0 = ptj
                        elif ki == 1:
                            if qlo > 0:
                                nc.vector.tensor_copy(rs[:, 0:qlo],
                                                      pt0[:, 0:qlo])
                            nc.vector.tensor_add(rs[:, qlo:QC],
                                                 pt0[:, qlo:QC],
                                                 ptj[:, qlo:QC])
                        else:
                            nc.vector.tensor_add(rs[:, qlo:QC],
                                                 rs[:, qlo:QC],
                                                 ptj[:, qlo:QC])
                        pend.append((ki, qlo, ptj))
                    while len(pend) > 4:
                        k0, q0, p0 = pend.pop(0)
                        nc.tensor.matmul(yps[:, q0:QC], v_sb[k0][:],
                                         p0[:, q0:QC], start=(k0 == 0),
                                         stop=(k0 == nki_hi - 1))
                    if kp == 0 and prev is not None:
                        denom_norm(sp, *prev)
                        prev = None
                for k0, q0, p0 in pend:
                    nc.tensor.matmul(yps[:, q0:QC], v_sb[k0][:], p0[:, q0:QC],
                                     start=(k0 == 0), stop=(k0 == nki_hi - 1))
                prev = (h, rs, yps, yT)
                # PE filler between heads: prior pass's out-proj rows and the
                # next pass's projection chains
                if sp > 0:
                    out_proj_chunk(sp - 1, h)
                if sp + 1 < NSP:
                    proj_group(sp + 1, h, qT_cur, xt_cur[0], xt_cur[1], ropes)
            denom_norm(sp, *prev)
            for args in ropes:
                rope_apply(*args)

        for loc in range(QC // 128):
            out_proj_chunk(NSP - 1, loc)

    nc.compile()
    return nc


def make_consts(S, QC=512):
    """Host-precomputed constant tensors (rope table, causal mask, sign, identity)."""
    rope_dim = D // 2
    j = np.arange(rope_dim, dtype=np.float64)
    thetas = 1.0 / ROPE_BASE ** (2.0 * j / rope_dim)
    positions = np.arange(S, dtype=np.float64)
    angles = positions[:, None] * thetas[None, :]
    sin = np.sin(np.concatenate([angles, angles], axis=1)).astype(np.float32)  # [S, D]
    sinT = np.ascontiguousarray(sin.T)                                          # [D, S]
    sgn = np.where(np.arange(D) < rope_dim, -1.0, 1.0).astype(np.float32)

    B = QC - 128
    MW = 2 * QC - 128
    k_idx = np.arange(128)[:, None]
    c_idx = np.arange(MW)[None, :]
    mw = (k_idx <= (c_idx - B)).astype(np.float32)

    cst = np.concatenate([mw[:, B:B + 128], np.eye(128, dtype=np.float32),
                          sgn.reshape(128, 1)], axis=1)
    return {
        "sinT": np.ascontiguousarray(sinT.astype(NP_BF16)),
        "cst": np.ascontiguousarray(cst).astype(NP_BF16),
    }


def _swz_w(W):
    """[A*128, N] -> SBUF layout [128, A*N] (chunk-major along free dim)."""
    A = W.shape[0] // 128
    return np.ascontiguousarray(
        W.reshape(A, 128, W.shape[1]).transpose(1, 0, 2).reshape(128, -1)
    ).astype(NP_BF16)


def _swz_x(xb, S, E, PW):
    """[S, E] -> [128, NSP*2*(EC/2)*PW]: per pass-half, e-chunk-major tiles."""
    NSP, EC = S // PW, E // 128
    a = xb.reshape(NSP, PW, EC, 128).transpose(3, 0, 2, 1)   # [128,NSP,EC,PW]
    a = a.reshape(128, NSP, 2, (EC // 2) * PW)
    return np.ascontiguousarray(a.reshape(128, -1)).astype(NP_BF16)


def make_in_maps(x, Wq, Wk, Wv, Wo, S, E, QC=512, bo=None):
    """Shard full inputs into the 8 per-core input maps (bf16 on-device)."""
    consts = make_consts(S, QC)
    EC_ = E // 128
    in_maps = []
    for c in range(N_CORES):
        b, g = c // TP, c % TP
        m = dict(consts)
        m["xh"] = _swz_x(x[b], S, E, QC)
        wqg = Wq[:, NHL * D * g:NHL * D * (g + 1)]            # [E, NHL*D]
        wqg = wqg.reshape(EC_, 128, NHL, D).transpose(1, 2, 0, 3)  # [128,h,e,d]
        m["Wq"] = np.ascontiguousarray(
            wqg.reshape(128, -1)).astype(NP_BF16)
        m["Wk"] = _swz_w(Wk[:, D * g:D * (g + 1)])
        m["Wv"] = _swz_w(Wv[:, D * g:D * (g + 1)])
        m["Wo"] = _swz_w(Wo[NHL * D * g:NHL * D * (g + 1), :])
        in_maps.append(m)
    return in_maps


_CACHE = {}


def _compiled_full():
    if "nc" not in _CACHE:
        _CACHE["nc"] = build_program(S_FULL, E_FULL)
    return _CACHE["nc"]


def kernel(x, Wq, Wk, Wv, Wo, bo):
    nc = _compiled_full()
    in_maps = make_in_maps(x, Wq, Wk, Wv, Wo, S_FULL, E_FULL, bo=bo)
    res = run_bass_kernel_spmd(nc, in_maps, list(range(N_CORES)))
    # unshard the row-parallel out-proj: sum the 4 head-group partials,
    # then add the output bias (host side of the reduce)
    out = np.zeros((BATCH, S_FULL, E_FULL), np.float32)
    for c in range(N_CORES):
        out[c // TP] += res.results[c]["out"].astype(np.float32)
    out += bo.astype(np.float32)[None, None, :]
    return out
